# revision 22
# baseline (speedup 1.0000x reference)
"""Trainium2 Bass kernel for nn_AdaptiveVAE (8-core data-parallel)."""
import os
import sys

sys.path.insert(0, "/opt/trn_rl_repo")

import numpy as np

import concourse.bass as bass
import concourse.tile as tile
from concourse import mybir, bacc

F32 = mybir.dt.float32

B, S, D, H, HD, FF, L, NF, NC, DH = 262144, 6, 16, 4, 4, 64, 10, 6, 139, 32
EPS = 1e-5
NCORES = 8
R = B // NCORES          # rows per core
TILE = 512               # rows per tile
CH = TILE // 128         # chunks per tile
P = 128

A = mybir.ActivationFunctionType
OP = mybir.AluOpType


def _consts(i):
    """Pack all weight-derived constant matrices (numpy f32)."""
    te = np.asarray(i["type_embed"], np.float32)
    vw = np.asarray(i["value_w"], np.float32)[:, 0]
    vb = np.asarray(i["value_b"], np.float32)
    pe = np.asarray(i["pos_enc"], np.float32)
    qkv_w = np.asarray(i["qkv_w"], np.float32)
    qkv_b = np.asarray(i["qkv_b"], np.float32)
    out_w = np.asarray(i["out_w"], np.float32)
    out_b = np.asarray(i["out_b"], np.float32)
    w1 = np.asarray(i["ffn_w1"], np.float32)
    b1 = np.asarray(i["ffn_b1"], np.float32)
    w2 = np.asarray(i["ffn_w2"], np.float32)
    b2 = np.asarray(i["ffn_b2"], np.float32)
    pool_w = np.asarray(i["pool_w"], np.float32)
    pool_b = np.asarray(i["pool_b"], np.float32)
    mu_w = np.asarray(i["mu_w"], np.float32)
    mu_b = np.asarray(i["mu_b"], np.float32)
    lv_w = np.asarray(i["lv_w"], np.float32)
    lv_b = np.asarray(i["lv_b"], np.float32)
    dw1 = np.asarray(i["dec_w1"], np.float32)
    db1 = np.asarray(i["dec_b1"], np.float32)
    dw2 = np.asarray(i["dec_w2"], np.float32)
    db2 = np.asarray(i["dec_b2"], np.float32)
    cw1 = np.asarray(i["cls_w1"], np.float32)
    cb1 = np.asarray(i["cls_b1"], np.float32)
    cw2 = np.asarray(i["cls_w2"], np.float32)
    cb2 = np.asarray(i["cls_b2"], np.float32)

    c = {}
    emb_w = np.zeros((42, 96), np.float32)
    for s in range(6):
        for e in range(6):
            emb_w[s * 6 + e, s * 16:(s + 1) * 16] = te[e]
        emb_w[36 + s, s * 16:(s + 1) * 16] = vw
    c["emb_w"] = emb_w
    c["emb_bias"] = (pe[:6] + vb[None, :]).reshape(96, 1).astype(np.float32)
    c["iota36"] = np.tile(
        np.tile(np.arange(6, dtype=np.float32), 6).reshape(1, 36), (P, 1))

    for l in range(2):
        wqkv = np.zeros((97, 288), np.float32)
        wproj = np.zeros((96, 96), np.float32)
        rproj = np.zeros((97, 96), np.float32)
        w1bl = np.zeros((96, 384), np.float32)
        w2bl = np.zeros((384, 96), np.float32)
        Wl = qkv_w[l]
        for s in range(6):
            sl = slice(s * 16, (s + 1) * 16)
            wqkv[sl, s * 16:(s + 1) * 16] = Wl[0:16].T
            wqkv[sl, 96 + s * 16:96 + (s + 1) * 16] = Wl[16:32].T
            wqkv[sl, 192 + s * 16:192 + (s + 1) * 16] = Wl[32:48].T
            wproj[sl, sl] = out_w[l].T
            w1bl[sl, s * 64:(s + 1) * 64] = w1[l].T
            w2bl[s * 64:(s + 1) * 64, sl] = w2[l].T
        wqkv[96, 0:96] = np.tile(qkv_b[l][0:16], 6)
        wqkv[96, 96:192] = np.tile(qkv_b[l][16:32], 6)
        wqkv[96, 192:288] = np.tile(qkv_b[l][32:48], 6)
        rproj[0:96, :] = np.eye(96, dtype=np.float32)
        rproj[96, :] = np.tile(out_b[l], 6)
        r2bl = np.zeros((97, 96), np.float32)
        r2bl[0:96, :] = np.eye(96, dtype=np.float32)
        r2bl[96, :] = np.tile(b2[l], 6)
        c[f"wqkv{l}"] = wqkv
        c[f"wproj{l}"] = wproj
        c[f"rproj{l}"] = rproj
        c[f"w1b{l}"] = w1bl
        c[f"b1c{l}"] = np.tile(b1[l], 6).reshape(3, 128).T.copy()
        for j in range(3):
            c[f"w2c{l}{j}"] = w2bl[j * 128:(j + 1) * 128, :].copy()
        c[f"r2b{l}"] = r2bl

    lnsum_bc = np.zeros((96, 96), np.float32)
    for s in range(6):
        lnsum_bc[s * 16:(s + 1) * 16, s * 16:(s + 1) * 16] = 1.0 / 16.0
    c["lnsum_bc"] = lnsum_bc

    pc = np.zeros((97, 16), np.float32)
    for s in range(6):
        pc[s * 16:(s + 1) * 16, :] = pool_w.T / 6.0
    pc[96, :] = pool_b
    c["pool_cat"] = pc

    wm = np.zeros((17, 42), np.float32)
    wm[0:16, 0:10] = mu_w.T
    wm[0:16, 32:42] = lv_w.T
    wm[16, 0:10] = mu_b
    wm[16, 32:42] = lv_b
    c["wmuvl"] = wm

    dc1a = np.zeros((11, 128), np.float32)
    dc1b = np.zeros((11, 96), np.float32)
    for e in range(4):
        dc1a[0:10, e * 32:(e + 1) * 32] = dw1[e].T
        dc1a[10, e * 32:(e + 1) * 32] = db1[e]
    for e in range(2):
        dc1b[0:10, e * 32:(e + 1) * 32] = dw1[4 + e].T
        dc1b[10, e * 32:(e + 1) * 32] = db1[4 + e]
    dc1b[0:10, 64:96] = cw1.T
    dc1b[10, 64:96] = cb1
    c["dc1a"] = dc1a
    c["dc1b"] = dc1b

    allo1 = np.zeros((128, 6), np.float32)
    allo2 = np.zeros((65, 6), np.float32)
    for e in range(4):
        allo1[e * 32:(e + 1) * 32, e] = dw2[e, 0]
    for e in range(2):
        allo2[e * 32:(e + 1) * 32, 4 + e] = dw2[4 + e, 0]
    allo2[64, :] = db2[:, 0]
    c["allo1"] = allo1
    c["allo2"] = allo2

    cwa = np.zeros((33, 128), np.float32)
    cwb = np.zeros((33, 11), np.float32)
    cwa[0:32, :] = cw2[0:128].T
    cwa[32, :] = cb2[0:128]
    cwb[0:32, :] = cw2[128:139].T
    cwb[32, :] = cb2[128:139]
    c["clsw2a"] = cwa
    c["clsw2b"] = cwb
    c["ident"] = np.eye(P, dtype=np.float32)
    c["eps96"] = np.full((96, 1), EPS, np.float32)
    return c


def _blob_layout(c):
    keys = sorted(c.keys())
    layout = {}
    col = 0
    for k in keys:
        r, w = c[k].shape
        layout[k] = (col, r, w)
        col += w
    return keys, layout, col


def _pack_blob(c):
    keys, layout, ncols = _blob_layout(c)
    blob = np.zeros((P, ncols), np.float32)
    for k in keys:
        col, r, w = layout[k]
        blob[0:r, col:col + w] = c[k]
    return blob, layout, ncols


def _ap(base, dims):
    """AP with base's partition dim + explicit (step, count) free dims."""
    return bass.AP(
        tensor=base.tensor,
        offset=base.offset,
        ap=[list(base.ap[0])] + [[s, n] for s, n in dims],
    )


def _blob_shapes():
    """Blob layout derived from dummy consts (shapes only)."""
    dummy = {k: np.zeros(sh, np.float32) for k, sh in [
        ("type_embed", (6, 16)), ("value_w", (16, 1)), ("value_b", (16,)),
        ("pos_enc", (6, 16)), ("qkv_w", (2, 48, 16)), ("qkv_b", (2, 48)),
        ("out_w", (2, 16, 16)), ("out_b", (2, 16)),
        ("ffn_w1", (2, 64, 16)), ("ffn_b1", (2, 64)),
        ("ffn_w2", (2, 16, 64)), ("ffn_b2", (2, 16)),
        ("pool_w", (16, 16)), ("pool_b", (16,)),
        ("mu_w", (10, 16)), ("mu_b", (10,)),
        ("lv_w", (10, 16)), ("lv_b", (10,)),
        ("dec_w1", (6, 32, 10)), ("dec_b1", (6, 32)),
        ("dec_w2", (6, 1, 32)), ("dec_b2", (6, 1)),
        ("cls_w1", (32, 10)), ("cls_b1", (32,)),
        ("cls_w2", (139, 32)), ("cls_b2", (139,))]}
    c = _consts(dummy)
    return _blob_layout(c)


def build_nc(rows, reps=1):
    n_tiles = rows // TILE
    _, layout, ncols = _blob_shapes()
    nc = bacc.Bacc("TRN2", target_bir_lowering=False, debug=False,
                   num_devices=NCORES)
    ids_d = nc.declare_dram_parameter("ids", [rows, 6], F32, isOutput=False)
    fv_d = nc.declare_dram_parameter("fv", [rows, 6], F32, isOutput=False)
    eps_d = nc.declare_dram_parameter("epsT", [10, rows], F32, isOutput=False)
    cb_d = nc.declare_dram_parameter("cblob", [P, ncols], F32, isOutput=False)
    rec_o = nc.declare_dram_parameter("rec", [rows, 6], F32, isOutput=True)
    mlv_o = nc.declare_dram_parameter("mlvT", [42, rows], F32, isOutput=True)
    lg_o = nc.declare_dram_parameter("lgT", [139, rows], F32, isOutput=True)

    with tile.TileContext(nc) as tc:
        with (
            tc.tile_pool(name="consts", bufs=1) as consts,
            tc.tile_pool(name="io", bufs=6) as io,
            tc.tile_pool(name="xf", bufs=10) as xf,
            tc.tile_pool(name="yf", bufs=3) as yf,
            tc.tile_pool(name="attn", bufs=3) as attn,
            tc.tile_pool(name="attnB", bufs=2) as attnB,
            tc.tile_pool(name="small", bufs=2) as small,
            tc.tile_pool(name="psA", bufs=2, space="PSUM") as psA,
            tc.tile_pool(name="psKV", bufs=1, space="PSUM") as psKV,
            tc.tile_pool(name="psB", bufs=2, space="PSUM") as psB,
            tc.tile_pool(name="psC", bufs=2, space="PSUM") as psC,
        ):
            blob = consts.tile([P, ncols], F32)
            nc.gpsimd.dma_start(blob[:], cb_d[:, :])

            def CT(k):
                col, r, w = layout[k]
                return blob[0:r, col:col + w]

            ident = CT("ident")
            eps96 = CT("eps96")

            for it in range(n_tiles * reps):
                r0 = (it % n_tiles) * TILE
                # ================= embed =================
                idst = io.tile([P, CH, 6], F32, tag="idst")
                nc.sync.dma_start(
                    idst[:],
                    ids_d[r0:r0 + TILE, :].rearrange("(c p) s -> p c s", p=P))
                stg = io.tile([P, CH, 48], F32, tag="stg")
                nc.sync.dma_start(
                    stg[:, :, 36:42],
                    fv_d[r0:r0 + TILE, :].rearrange("(c p) s -> p c s", p=P))
                # one-hot: stg[:, c, s*6+e] = (ids[c,s]==e)
                nc.vector.tensor_tensor(
                    _ap(stg[:, :, 0:36], [(48, CH), (6, 6), (1, 6)]),
                    _ap(idst[:], [(6, CH), (1, 6), (0, 6)]),
                    _ap(CT("iota36"), [(0, CH), (6, 6), (1, 6)]),
                    op=OP.is_equal)
                # transpose staging -> stF [42, 512]
                stF = io.tile([42, TILE], F32, tag="stF")
                for cidx in range(CH):
                    tp = psA.tile([42, P], F32, tag="t")
                    nc.tensor.transpose(tp[:], stg[:, cidx, 0:42], ident)
                    nc.scalar.copy(stF[:, cidx * P:(cidx + 1) * P], tp[:])
                # x0 = emb_w.T @ stF + bias
                x0p = psB.tile([96, TILE], F32, tag="r")
                nc.tensor.matmul(x0p[:], CT("emb_w"), stF[:],
                                 start=True, stop=True)
                x = xf.tile([97, TILE], F32, tag="x")
                nc.scalar.activation(x[0:96, :], x0p[:], A.Identity,
                                     bias=CT("emb_bias"))
                nc.gpsimd.memset(x[96:97, :], 1.0)

                # ================= transformer layers =================
                for l in range(2):
                    oF = attn.tile([96, TILE], F32, tag="oF")
                    # q/k/v for all chunks: separate PSUM tiles [128, CH*96]
                    q_ps = psA.tile([P, CH * 96], F32, tag="t")
                    k_ps = psKV.tile([P, CH * 96], F32, tag="kps")
                    v_ps = psKV.tile([P, CH * 96], F32, tag="vps")
                    for cidx in range(CH):
                        xs = x[:, cidx * P:(cidx + 1) * P]
                        co = cidx * 96
                        nc.tensor.matmul(q_ps[:, co:co + 96], xs,
                                         CT(f"wqkv{l}")[:, 0:96],
                                         start=True, stop=True)
                        nc.tensor.matmul(k_ps[:, co:co + 96], xs,
                                         CT(f"wqkv{l}")[:, 96:192],
                                         start=True, stop=True)
                        nc.tensor.matmul(v_ps[:, co:co + 96], xs,
                                         CT(f"wqkv{l}")[:, 192:288],
                                         start=True, stop=True)
                    qs = attn.tile([P, CH * 96], F32, tag="qs")
                    nc.scalar.copy(qs[:], q_ps[:])
                    # scores products, all chunks: pr1 layout (c,i,j,h,d)
                    pr1 = attnB.tile([P, CH * 576], F32, tag="pr1")
                    nc.vector.tensor_tensor(
                        _ap(pr1[:], [(576, CH), (96, 6), (16, 6), (4, 4), (1, 4)]),
                        _ap(qs[:], [(96, CH), (16, 6), (0, 6), (4, 4), (1, 4)]),
                        _ap(k_ps[:], [(96, CH), (0, 6), (16, 6), (4, 4), (1, 4)]),
                        op=OP.mult)
                    # sc layout (c,i,j,h)
                    sc = attnB.tile([P, CH * 144], F32, tag="sc")
                    nc.vector.tensor_reduce(
                        sc[:], _ap(pr1[:], [(4, CH * 144), (1, 4)]),
                        op=OP.add, axis=mybir.AxisListType.X)
                    # exp + relayout to (c,i,h,j)
                    esc = attnB.tile([P, CH * 144], F32, tag="esc")
                    nc.scalar.activation(
                        _ap(esc[:], [(144, CH), (24, 6), (1, 6), (6, 4)]),
                        sc[:], A.Exp, scale=0.5)
                    den = attn.tile([P, CH * 24], F32, tag="den")
                    nc.vector.tensor_reduce(
                        den[:], _ap(esc[:], [(6, CH * 24), (1, 6)]),
                        op=OP.add, axis=mybir.AxisListType.X)
                    rcp = attn.tile([P, CH * 24], F32, tag="rcp")
                    nc.vector.reciprocal(rcp[:], den[:])
                    # attn-weighted V per chunk into pr2 (c | j,i,h,d)
                    pr2 = attnB.tile([P, CH * 576], F32, tag="pr2")
                    for cidx in range(CH):
                        nc.vector.tensor_tensor(
                            _ap(pr2[:, cidx * 576:(cidx + 1) * 576],
                                [(96, 6), (16, 6), (4, 4), (1, 4)]),
                            _ap(esc[:, cidx * 144:(cidx + 1) * 144],
                                [(1, 6), (24, 6), (6, 4), (0, 4)]),
                            _ap(v_ps[:, cidx * 96:(cidx + 1) * 96],
                                [(16, 6), (0, 6), (4, 4), (1, 4)]),
                            op=OP.mult)
                    oun = attn.tile([P, CH * 96], F32, tag="oun")
                    nc.vector.tensor_reduce(
                        oun[:], _ap(pr2[:], [(576, CH), (1, 96), (96, 6)]),
                        op=OP.add, axis=mybir.AxisListType.X)
                    ot = attn.tile([P, CH * 96], F32, tag="ot")
                    nc.vector.tensor_tensor(
                        ot[:], oun[:],
                        _ap(rcp[:], [(24, CH), (4, 6), (1, 4), (0, 4)]),
                        op=OP.mult)
                    for cidx in range(CH):
                        otp = psA.tile([96, P], F32, tag="t")
                        nc.tensor.transpose(
                            otp[:], ot[:, cidx * 96:(cidx + 1) * 96], ident)
                        nc.scalar.copy(oF[:, cidx * P:(cidx + 1) * P], otp[:])
                    # out proj + residual
                    rp = psB.tile([96, TILE], F32, tag="r")
                    nc.tensor.matmul(rp[:], CT(f"wproj{l}"), oF[:],
                                     start=True, stop=False)
                    nc.tensor.matmul(rp[:], CT(f"rproj{l}"), x[:],
                                     start=False, stop=True)
                    x = _layer_norm(nc, xf, yf, small, psC, CT, eps96, rp)
                    # FFN (accumulate into r2, residual via identity matmul)
                    r2 = psB.tile([96, TILE], F32, tag="r")
                    for j in range(3):
                        hp = psB.tile([P, TILE], F32, tag="r")
                        nc.tensor.matmul(
                            hp[:], CT(f"w1b{l}")[:, j * P:(j + 1) * P],
                            x[0:96, :], start=True, stop=True)
                        hj = attn.tile([P, TILE], F32, tag="hj")
                        nc.scalar.activation(
                            hj[:], hp[:], A.Relu,
                            bias=CT(f"b1c{l}")[:, j:j + 1])
                        nc.tensor.matmul(
                            r2[:], CT(f"w2c{l}{j}"), hj[:],
                            start=(j == 0), stop=False)
                    nc.tensor.matmul(r2[:], CT(f"r2b{l}"), x[:],
                                     start=False, stop=True)
                    x = _layer_norm(nc, xf, yf, small, psC, CT, eps96, r2)

                # ================= head =================
                plp = psC.tile([16, TILE], F32, tag="st")
                nc.tensor.matmul(plp[:], CT("pool_cat"), x[:],
                                 start=True, stop=True)
                plr = small.tile([17, TILE], F32, tag="plr")
                nc.gpsimd.memset(plr[:], 1.0)
                nc.scalar.activation(plr[0:16, :], plp[:], A.Relu)
                mlv = psC.tile([42, TILE], F32, tag="st")
                nc.tensor.matmul(mlv[:], CT("wmuvl"), plr[:],
                                 start=True, stop=True)
                mlvs = small.tile([42, TILE], F32, tag="mlvs")
                nc.scalar.copy(mlvs[:], mlv[:])
                nc.gpsimd.dma_start(mlv_o[:, r0:r0 + TILE], mlvs[:])
                estd = small.tile([10, TILE], F32, tag="estd")
                nc.scalar.activation(estd[:], mlv[32:42, :], A.Exp, scale=0.5)
                epst = small.tile([10, TILE], F32, tag="epst")
                nc.sync.dma_start(epst[:], eps_d[:, r0:r0 + TILE])
                ez = small.tile([10, TILE], F32, tag="ez")
                nc.vector.tensor_tensor(ez[:], estd[:], epst[:], op=OP.mult)
                z = small.tile([11, TILE], F32, tag="z")
                nc.gpsimd.memset(z[:], 1.0)
                nc.vector.tensor_tensor(z[0:10, :], mlv[0:10, :], ez[:],
                                        op=OP.add)
                # decoders + classifier hidden
                hc1 = psB.tile([P, TILE], F32, tag="r")
                nc.tensor.matmul(hc1[:], CT("dc1a"), z[:],
                                 start=True, stop=True)
                hc2 = psB.tile([96, TILE], F32, tag="r")
                nc.tensor.matmul(hc2[:], CT("dc1b"), z[:],
                                 start=True, stop=True)
                hr1 = attn.tile([P, TILE], F32, tag="hj")
                nc.scalar.activation(hr1[:], hc1[:], A.Relu)
                hr2 = small.tile([65, TILE], F32, tag="hr2")
                nc.gpsimd.memset(hr2[64:65, :], 1.0)
                nc.scalar.activation(hr2[0:64, :], hc2[0:64, :], A.Relu)
                hrc = small.tile([33, TILE], F32, tag="hrc")
                nc.gpsimd.memset(hrc[32:33, :], 1.0)
                nc.scalar.activation(hrc[0:32, :], hc2[64:96, :], A.Relu)
                # allo (per-expert scalar outputs)
                alp = psC.tile([6, TILE], F32, tag="st")
                nc.tensor.matmul(alp[:], CT("allo1"), hr1[:],
                                 start=True, stop=False)
                nc.tensor.matmul(alp[:], CT("allo2"), hr2[:],
                                 start=False, stop=True)
                als = small.tile([6, TILE], F32, tag="als")
                nc.scalar.copy(als[:], alp[:])
                # logits
                lg1 = psB.tile([P, TILE], F32, tag="r")
                nc.tensor.matmul(lg1[:], CT("clsw2a"), hrc[:],
                                 start=True, stop=True)
                lg2 = psC.tile([11, TILE], F32, tag="st")
                nc.tensor.matmul(lg2[:], CT("clsw2b"), hrc[:],
                                 start=True, stop=True)
                lg1s = attn.tile([P, TILE], F32, tag="hj")
                nc.scalar.copy(lg1s[:], lg1[:])
                lg2s = small.tile([11, TILE], F32, tag="lg2s")
                nc.scalar.copy(lg2s[:], lg2[:])
                nc.sync.dma_start(lg_o[0:128, r0:r0 + TILE], lg1s[:])
                nc.gpsimd.dma_start(lg_o[128:139, r0:r0 + TILE], lg2s[:])
                # recons: gather allo by feature id via one-hot
                rec_sb = small.tile([P, CH, 6], F32, tag="rec")
                for cidx in range(CH):
                    atp = psC.tile([P, 6], F32, tag="st")
                    nc.tensor.transpose(
                        atp[:], als[:, cidx * P:(cidx + 1) * P],
                        ident[0:6, 0:6])
                    prr = small.tile([P, 36], F32, tag="prr")
                    nc.vector.tensor_tensor(
                        prr[:], stg[:, cidx, 0:36],
                        _ap(atp[:], [(0, 6), (1, 6)]),
                        op=OP.mult)
                    nc.vector.tensor_reduce(
                        rec_sb[:, cidx, :], _ap(prr[:], [(6, 6), (1, 6)]),
                        op=OP.add, axis=mybir.AxisListType.X)
                nc.gpsimd.dma_start(
                    rec_o[r0:r0 + TILE, :].rearrange("(c p) s -> p c s", p=P),
                    rec_sb[:])

    nc.compile()
    return nc


def _layer_norm(nc, xf, yf, small, psC, CT, eps96, src_psum):
    """LayerNorm over d (16) per position, F layout. src_psum [96, TILE] PSUM.
    Returns new x tile [97, TILE] SBUF with ones row 96."""
    y = yf.tile([97, TILE], F32, tag="y")
    nc.scalar.copy(y[0:96, :], src_psum[:])
    nc.gpsimd.memset(y[96:97, :], 1.0)
    sq = small.tile([96, TILE], F32, tag="sq")
    nc.scalar.activation(sq[:], y[0:96, :], A.Square)
    mb = psC.tile([96, TILE], F32, tag="st")
    nc.tensor.matmul(mb[:], CT("lnsum_bc"), y[0:96, :], start=True, stop=True)
    sb = psC.tile([96, TILE], F32, tag="st")
    nc.tensor.matmul(sb[:], CT("lnsum_bc"), sq[:], start=True, stop=True)
    msq = small.tile([96, TILE], F32, tag="msq")
    nc.scalar.activation(msq[:], mb[:], A.Square)
    var = small.tile([96, TILE], F32, tag="var")
    nc.vector.tensor_tensor(var[:], sb[:], msq[:], op=OP.subtract)
    sd = small.tile([96, TILE], F32, tag="sd")
    nc.scalar.activation(sd[:], var[:], A.Sqrt, bias=eps96)
    rstd = small.tile([96, TILE], F32, tag="rstd")
    nc.vector.reciprocal(rstd[:], sd[:])
    t1 = small.tile([96, TILE], F32, tag="t1")
    nc.vector.tensor_tensor(t1[:], y[0:96, :], rstd[:], op=OP.mult)
    mrb = small.tile([96, TILE], F32, tag="mrb")
    nc.vector.tensor_tensor(mrb[:], mb[:], rstd[:], op=OP.mult)
    xn = xf.tile([97, TILE], F32, tag="x")
    nc.vector.tensor_tensor(xn[0:96, :], t1[:], mrb[:], op=OP.subtract)
    nc.gpsimd.memset(xn[96:97, :], 1.0)
    return xn


_NC_CACHE = {}


def _get_nc(rows, reps=1):
    key = (rows, reps)
    if key not in _NC_CACHE:
        _NC_CACHE[key] = build_nc(rows, reps)
    return _NC_CACHE[key]


def _gen_eps(rows_total):
    """jax.random.normal(key(42)) computed in a CPU-pinned subprocess."""
    import subprocess, tempfile
    fd, path = tempfile.mkstemp(suffix=".npy")
    os.close(fd)
    code = (
        "import sys, numpy as np, jax, jax.numpy as jnp\n"
        f"e = jax.random.normal(jax.random.key(42), ({rows_total}, 10), jnp.float32)\n"
        "np.save(sys.argv[1], np.asarray(e))\n"
    )
    env = dict(os.environ, JAX_PLATFORMS="cpu")
    env.pop("TRN_TERMINAL_POOL_IPS", None)
    env["PYTHONPATH"] = os.pathsep.join(p for p in sys.path if p)
    subprocess.run([sys.executable, "-c", code, path], env=env, check=True)
    eps = np.load(path)
    os.unlink(path)
    return eps


def _host_prep(inputs, rows_total):
    ids = np.asarray(inputs["feature_ids"], np.int32).astype(np.float32)
    fv = np.asarray(inputs["feature_values"], np.float32)[..., 0]
    eps = _gen_eps(rows_total)
    epsT = np.ascontiguousarray(eps.T)
    return ids, fv, epsT


def kernel(**inputs):
    from concourse.bass_utils import run_bass_kernel_spmd
    ids, fv, epsT = _host_prep(inputs, B)
    blob, _, _ = _pack_blob(_consts(inputs))
    nc = _get_nc(R)
    in_maps = []
    for i in range(NCORES):
        m = {
            "ids": np.ascontiguousarray(ids[i * R:(i + 1) * R]),
            "fv": np.ascontiguousarray(fv[i * R:(i + 1) * R]),
            "epsT": np.ascontiguousarray(epsT[:, i * R:(i + 1) * R]),
            "cblob": blob,
        }
        in_maps.append(m)
    res = run_bass_kernel_spmd(nc, in_maps, core_ids=list(range(NCORES)))
    rec = np.concatenate([res.results[i]["rec"] for i in range(NCORES)], 0)
    mlv = np.concatenate([res.results[i]["mlvT"] for i in range(NCORES)], 1)
    lg = np.concatenate([res.results[i]["lgT"] for i in range(NCORES)], 1).T
    mu = mlv[0:10].T
    lv = mlv[32:42].T
    return (rec[..., None].astype(np.float32),
            np.ascontiguousarray(mu, np.float32),
            np.ascontiguousarray(lv, np.float32),
            np.ascontiguousarray(lg, np.float32))
